# revision 13
# baseline (speedup 1.0000x reference)
"""Trainium2 Bass kernel for nn_ChannelLatencySeq2Seq.

Math (matching reference.py):
  - 3 depthwise convs (k=3,5,9; 6 outs each) + per-channel reduce over D=18
    collapse into ONE per-channel 9-tap FIR: Keff[c, tap].
  - LIF scan V_t = a*V + (1-a)*drive_t; latency = first t with V_t >= TH.
  - act = exp(-lat/scale); recon[b,j,t] = sum_c act[b,c]*G[j,c,t], where
    G = og[j,c] * sum_d fw[j,c,d]*kp[c,d,t] is nonzero only for t<9.

Sharding: data-parallel over batch B=16 across 8 cores (2 batches/core).
Per core the 512 (b,c) rows sit on 4 partition-tiles of 128; T=1024 on the
free axis.  Conv runs as 9 fused MACs (scalar_tensor_tensor) on DVE, the
scan as one tensor_tensor_scan per tile, thresholding via ACT Sign and a
fused multiply+max-reduce, and the reconstruction einsum as a small fp32
TensorE matmul against host-packed G.
"""

import sys
import numpy as np

if "/opt/trn_rl_repo" not in sys.path:
    sys.path.insert(0, "/opt/trn_rl_repo")

B, C, T = 16, 256, 1024
KERNEL_SPECS = [(3, 6), (5, 6), (9, 6)]
D = 18
TAU = 5.0
ALPHA = float(np.exp(-1.0 / TAU))
THRESHOLD = 0.01
NCORES = 8
BL = B // NCORES          # batches per core = 2
ROWS = BL * C             # 512 rows per core
NTILES = ROWS // 128      # 4
KT = 9                    # effective taps
PAD = 4
JT = C * KT               # 2304 recon columns (j major, t minor)

_compiled = None
last_results = None       # BassKernelResults of most recent run (for test.py)


def _build():
    import os
    import concourse.bass as bass
    import concourse.mybir as mybir
    from concourse import bacc
    from concourse.tile import TileContext

    LEVEL = int(os.environ.get("DEBUG_LEVEL", "5"))
    f32 = mybir.dt.float32
    Alu = mybir.AluOpType
    Act = mybir.ActivationFunctionType

    nc = bacc.Bacc(None, target_bir_lowering=False)

    x_ext = nc.declare_dram_parameter("x", [ROWS, T], f32, isOutput=False)
    keff_ext = nc.declare_dram_parameter("keff", [ROWS, KT], f32, isOutput=False)
    rb_ext = nc.declare_dram_parameter("rb", [ROWS, 1], f32, isOutput=False)
    g_ext = nc.declare_dram_parameter("g", [C, JT], f32, isOutput=False)
    riota_ext = nc.declare_dram_parameter("riota", [1, T], f32, isOutput=False)
    sc_ext = nc.declare_dram_parameter("sc", [1, 1], f32, isOutput=False)

    lat_ext = nc.declare_dram_parameter("lat_o", [128, NTILES], f32, isOutput=True)
    act_ext = nc.declare_dram_parameter("act_o", [2, 128, BL], f32, isOutput=True)
    recon_ext = nc.declare_dram_parameter("recon_o", [BL, JT], f32, isOutput=True)

    with TileContext(nc) as tc:
        with (
            tc.tile_pool(name="const", bufs=1) as const,
            tc.tile_pool(name="work", bufs=2) as work,
            tc.tile_pool(name="psum", bufs=1, space="PSUM") as psum,
        ):
            # ---- constants ----
            alphaT = const.tile([128, T], f32, tag="alphaT")
            nc.gpsimd.memset(alphaT[:, :], ALPHA)
            zeroT = const.tile([128, T], f32, tag="zeroT")
            nc.gpsimd.memset(zeroT[:, :], 0.0)
            scT = const.tile([128, 1], f32, tag="scT")
            nc.gpsimd.dma_start(out=scT[:, :], in_=sc_ext[:, :].to_broadcast([128, 1]))
            thT = const.tile([128, 1], f32, tag="thT")
            nc.gpsimd.memset(thT[:, :], -THRESHOLD)

            g_sb = []
            for h in range(2):
                gt = const.tile([128, JT], f32, tag=f"g{h}", name=f"g{h}")
                nc.sync.dma_start(out=gt[:, :], in_=g_ext[128 * h:128 * (h + 1), :])
                g_sb.append(gt)

            lat_pack = const.tile([128, NTILES], f32, tag="lat_pack")
            actT = [const.tile([128, BL], f32, tag=f"actT{h}", name=f"actT{h}") for h in range(2)]

            # ---- per-tile pipeline ----
            for i in range(NTILES):
                r0 = 128 * i
                keff = const.tile([128, KT], f32, tag=f"keff{i}", name=f"keff{i}")
                nc.sync.dma_start(out=keff[:, :], in_=keff_ext[r0:r0 + 128, :])
                rb = const.tile([128, 1], f32, tag=f"rb{i}", name=f"rb{i}")
                nc.sync.dma_start(out=rb[:, :], in_=rb_ext[r0:r0 + 128, :])

                xpad = work.tile([128, T + 2 * PAD], f32, tag="xpad")
                nc.gpsimd.memset(xpad[:, 0:PAD], 0.0)
                nc.gpsimd.memset(xpad[:, T + PAD:T + 2 * PAD], 0.0)
                nc.sync.dma_start(out=xpad[:, PAD:PAD + T], in_=x_ext[r0:r0 + 128, :])

                if LEVEL < 2:
                    nc.vector.tensor_scalar(
                        out=lat_pack[:, i:i + 1], in0=xpad[:, 100:101],
                        scalar1=1.0, scalar2=None, op0=Alu.mult)
                    continue

                # conv: drive = sum_tap keff[:, tap] * x[t + tap - 4] + rb
                drive = work.tile([128, T], f32, tag="drive")
                nc.vector.tensor_scalar(
                    out=drive[:, :], in0=xpad[:, 0:T],
                    scalar1=keff[:, 0:1], scalar2=rb[:, 0:1],
                    op0=Alu.mult, op1=Alu.add)
                for tap in range(1, KT):
                    nc.vector.scalar_tensor_tensor(
                        out=drive[:, :], in0=xpad[:, tap:tap + T],
                        scalar=keff[:, tap:tap + 1], in1=drive[:, :],
                        op0=Alu.mult, op1=Alu.add)

                if LEVEL < 3:
                    nc.vector.tensor_scalar(
                        out=lat_pack[:, i:i + 1], in0=drive[:, 100:101],
                        scalar1=1.0, scalar2=None, op0=Alu.mult)
                    continue

                # LIF scan: V_t = alpha*V_{t-1} + drive'_t
                vtile = work.tile([128, T], f32, tag="vtile")
                nc.vector.tensor_tensor_scan(
                    out=vtile[:, :], data0=alphaT[:, :], data1=drive[:, :],
                    initial=0.0, op0=Alu.mult, op1=Alu.add)

                if LEVEL < 4:
                    nc.vector.tensor_scalar(
                        out=lat_pack[:, i:i + 1], in0=vtile[:, 100:101],
                        scalar1=1.0, scalar2=None, op0=Alu.mult)
                    continue

                # running max M_t = max_{s<=t} V_s  (monotone in t)
                mtile = work.tile([128, T], f32, tag="mtile")
                nc.vector.tensor_tensor_scan(
                    out=mtile[:, :], data0=vtile[:, :], data1=zeroT[:, :],
                    initial=-1e30, op0=Alu.max, op1=Alu.add)

                # s2 = sum_t Sign(M_t - TH) = (T - lat) - lat  ->  lat = (T - s2)/2
                sgn = work.tile([128, T], f32, tag="sgn")
                nc.scalar.activation(
                    out=sgn[:, :], in_=mtile[:, :], func=Act.Sign,
                    bias=thT[:, 0:1], scale=1.0,
                    accum_out=lat_pack[:, i:i + 1])
                if LEVEL == 35:
                    continue
                # lat_pack currently holds s2; turn into lat in place
                nc.vector.tensor_scalar(
                    out=lat_pack[:, i:i + 1], in0=lat_pack[:, i:i + 1],
                    scalar1=-0.5, scalar2=float(T) / 2.0, op0=Alu.mult, op1=Alu.add)

                # act = exp(-lat/scale) -> column of actT[h], h = i % 2, b = i // 2
                h, b = i % 2, i // 2
                nc.scalar.activation(
                    out=actT[h][:, b:b + 1], in_=lat_pack[:, i:i + 1],
                    func=Act.Exp, bias=0.0, scale=scT[:, 0:1])

            if LEVEL < 4 or LEVEL in (35, 37):
                for h in range(2):
                    nc.vector.memset(actT[h][:, :], 0.0)

            # ---- recon: out[b, (j,t)] = sum_c act[c, b] * G[c, (j,t)] ----
            recon_sb = const.tile([BL, JT], f32, tag="recon_sb")
            if LEVEL == 5:
                nchunks = [(n0, min(512, JT - n0)) for n0 in range(0, JT, 512)]
                ps = []
                for ci, (n0, nn) in enumerate(nchunks):
                    pt = psum.tile([BL, nn], mybir.dt.float32, tag=f"ps{ci}", name=f"ps{ci}")
                    ps.append(pt)
                    for h in range(2):
                        nc.tensor.matmul(
                            out=pt[:, :], lhsT=actT[h][:, :], rhs=g_sb[h][:, n0:n0 + nn],
                            start=(h == 0), stop=(h == 1))
                for ci, (n0, nn) in enumerate(nchunks):
                    nc.scalar.activation(
                        out=recon_sb[:, n0:n0 + nn], in_=ps[ci][:, :],
                        func=Act.Copy, bias=0.0, scale=1.0)
            else:
                nc.vector.memset(recon_sb[:, :], 0.0)

            # ---- outputs ----
            nc.sync.dma_start(out=lat_ext[:, :], in_=lat_pack[:, :])
            for h in range(2):
                nc.sync.dma_start(out=act_ext[h], in_=actT[h][:, :])
            nc.sync.dma_start(out=recon_ext[:, :], in_=recon_sb[:, :])

    nc.compile()
    return nc


def _host_prep(inputs):
    """Host-side packing of weight-derived constants (no x-dependent math)."""
    gi = lambda k: np.asarray(inputs[k], np.float32)
    x = gi("x")
    rw = gi("reduce_w")            # (C, D)
    rbv = gi("reduce_b")           # (C,)
    og = gi("output_gates")        # (C, C)
    fw = gi("filter_weights")      # (C, C, D)
    ls = float(np.asarray(inputs["latency_scale"], np.float32))

    ws = {k: gi(f"w{k}").reshape(C, op, k) for k, op in KERNEL_SPECS}

    # Keff[c, tap]  (tap index 0..8 maps to offset tap-4)
    keff = np.zeros((C, KT), np.float64)
    off = 0
    for k, op in KERNEL_SPECS:
        p = (k - 1) // 2
        w = ws[k].astype(np.float64)                      # (C, op, k)
        # conv output uses x[t - p + i] * w[..., i] -> tap = i - p
        for i in range(k):
            tap = i - p + PAD
            keff[:, tap] += (w[:, :, i] * rw[:, off:off + op].astype(np.float64)).sum(axis=1)
        off += op
    keff *= (1.0 - ALPHA)
    keff = keff.astype(np.float32)                        # (C, 9)

    # conv biases flow through the reduce einsum: rb_eff = reduce_b + sum_d rw*b_d
    biases = np.concatenate(
        [np.asarray(inputs[f"b{k}"], np.float32).reshape(C, op) for k, op in KERNEL_SPECS],
        axis=1)                                           # (C, D)
    rb_eff = rbv.astype(np.float64) + (rw.astype(np.float64) * biases.astype(np.float64)).sum(axis=1)
    rb2 = ((1.0 - ALPHA) * rb_eff).astype(np.float32)     # (C,)

    # G[c, j*9+t] = og[j,c] * sum_d fw[j,c,d] * kp[c,d,t]
    kp = np.zeros((C, D, KT), np.float64)
    off = 0
    for k, op in KERNEL_SPECS:
        kp[:, off:off + op, :k] = ws[k].astype(np.float64)
        off += op
    gjct = np.einsum("jcd,cdt->jct", og.astype(np.float64)[:, :, None] * fw.astype(np.float64), kp)
    gmat = np.ascontiguousarray(gjct.transpose(1, 0, 2).reshape(C, JT)).astype(np.float32)

    riota = (float(T) - np.arange(T, dtype=np.float64)).astype(np.float32).reshape(1, T)
    scale = max(ls, 0.001)
    sc = np.asarray([[-1.0 / scale]], np.float32)

    keff_rows = np.tile(keff, (BL, 1))                     # (512, 9)
    rb_rows = np.tile(rb2.reshape(C, 1), (BL, 1))          # (512, 1)

    in_maps = []
    for core in range(NCORES):
        xs = np.ascontiguousarray(x[BL * core: BL * (core + 1)].reshape(ROWS, T))
        in_maps.append(dict(x=xs, keff=keff_rows, rb=rb_rows, g=gmat,
                            riota=riota, sc=sc))
    return in_maps


def kernel(**inputs):
    global _compiled, last_results
    from concourse.bass_utils import run_bass_kernel_spmd

    x = np.asarray(inputs["x"], np.float32)
    in_maps = _host_prep(inputs)

    if _compiled is None:
        _compiled = _build()
    nc = _compiled

    res = run_bass_kernel_spmd(nc, in_maps, list(range(NCORES)))
    last_results = res

    lat = np.empty((B, C), np.float32)
    act = np.empty((B, C), np.float32)
    recon = np.zeros((B, C, T), np.float32)
    for core in range(NCORES):
        r = res.results[core]
        lat_np = r["lat_o"]                      # (128, 4): col i = tile i
        act_np = r["act_o"]                      # (2, 128, BL)
        rec_np = r["recon_o"]                    # (BL, JT)
        lat_rows = lat_np.T.reshape(ROWS)        # rows (b*C + c)
        lat[BL * core: BL * (core + 1)] = lat_rows.reshape(BL, C)
        for b in range(BL):
            for h in range(2):
                act[BL * core + b, 128 * h:128 * (h + 1)] = act_np[h, :, b]
        recon[BL * core: BL * (core + 1), :, :KT] = rec_np.reshape(BL, C, KT)
    return recon, x, lat, act
